# revision 4
# baseline (speedup 1.0000x reference)
"""Self-contained TRN2 Bass kernel for nn_EuclideanSimilarity.

Full-input contract: kernel(x, W, b) with
  x [4, 4096, 128] f32, W [128, 128] f32, b [128] f32
returns out [4, 4096, 4096] f32 = exp(-pairwise_euclidean_dist(x @ W.T + b)).

The output is symmetric per batch, so each core computes only the circulant
upper half: query row-block i (128 rows) against key blocks i..i+16 (mod 32),
2176 columns per row-block. Every unordered pair is covered exactly once
(d = |i-j| mod 32 in [1,15] once, d=16 from both sides, d=0 the diagonal);
the host mirrors the remaining residues with block transposes at unshard
time. 8 cores: core c -> (batch c//2, half c%2); the host rotates the key
axis by 2048*half so all cores run an IDENTICAL program on local qtiles
0..15 with key columns [128q, 128q+2176) - no wraparound on device.

Per-core pipeline (engines balanced, ~one pass each):
  PE    gram matmuls, single fp32r pass (lhsT = -2h slice, rhs = h)
  DVE   fused drain: d2 = (psum + |h_q|^2) + |h_k|^2-broadcast, one
        scalar_tensor_tensor per PSUM tile (PSUM readable only by DVE/Act)
  Pool  d = d2^0.5 via tensor_tensor pow (GPSIMD cannot touch PSUM, but
        pow is only ISA-valid on GPSIMD - so the drain and the sqrt trade
        engines), in-place on the staging tile
  Act   out_bf16 = Exp(-d); Identity+Exp live in one activation-table set,
        so there are no mid-kernel table swaps
  DVE   diagonal-block fix: (out min 1.0) max I. fp32r noise makes the
        diagonal d2 ~ +-0.02 (sqrt of a negative -> NaN); min/max are IEEE
        minNum/maxNum on-chip, so NaN and any >1 garbage both collapse to
        the exact 1.0 the diagonal wants.
  DMA   one [128, 2176] bf16 store per qtile (~9.5 MB/core instead of 32)

The bf16 store + half-sized compute put every engine near ~30-45us; fp32r
gram noise (~2^-13) only matters on the diagonal, which the fix rewrites.
"""

from contextlib import ExitStack

import numpy as np

import concourse.mybir as mybir
import concourse.tile as tile
from concourse import bacc
from concourse.bass import ts
from concourse.masks import make_identity

F32 = mybir.dt.float32
F32R = mybir.dt.float32r
BF16 = mybir.dt.bfloat16
AF = mybir.ActivationFunctionType
ALU = mybir.AluOpType

B = 4
N = 4096
D = 128
NQ = 2048   # local query rows per core
NQT = 16    # local query tiles (128 rows each)
WQ = 2176   # key columns per qtile: 17 x 128 blocks
N_CORES = 8
TEMPERATURE = 1.0


def kernel_body(ctx: ExitStack, tc: tile.TileContext, out, xk, W, b):
    nc = tc.nc

    consts = ctx.enter_context(tc.tile_pool(name="consts", bufs=1))
    ident = consts.tile([128, 128], F32)
    make_identity(nc, ident[:])
    ident_bf = consts.tile([128, 128], BF16)
    nc.vector.tensor_copy(ident_bf[:], ident[:])

    w_sb = consts.tile([128, 128], F32)
    nc.sync.dma_start(w_sb[:], W[:, :])
    b_sb = consts.tile([128, 1], F32)
    nc.sync.dma_start(b_sb[:], b[:, :])
    bm2_sb = consts.tile([128, 1], F32)
    nc.scalar.mul(bm2_sb[:], b_sb[:], -2.0)

    ones_f = consts.tile([128, 128], F32)
    nc.gpsimd.memset(ones_f[:], 1.0)
    ones_full = consts.tile([128, 128], F32R)
    nc.vector.tensor_copy(ones_full[:], ones_f[:])

    half_tile = consts.tile([128, WQ], F32)
    nc.vector.memset(half_tile[:], 0.5)

    # persistent operands
    hpool = ctx.enter_context(tc.tile_pool(name="h", bufs=1))
    hk = hpool.tile([128, N], F32R)        # h keys, d-major
    gq = hpool.tile([128, NQ], F32R)       # -2h for local queries
    sqk_bc = hpool.tile([128, N], F32)     # |h_k|^2 broadcast across partitions
    sqq_cols = hpool.tile([128, NQT], F32)  # |h_q|^2, one col per qtile

    xk_r = xk.rearrange("(t p) d -> p t d", p=128)

    # ---------------- setup: project keys, norms ----------------
    with tc.tile_pool(name="setup_sb", bufs=3) as ssb, \
         tc.tile_pool(name="setup_ps", bufs=2, space="PSUM") as sps:

        wt_ps = sps.tile([128, 512], F32, tag="wt", bufs=1)
        nc.tensor.transpose(wt_ps[:, 0:128], w_sb[:], ident[:])
        wt_sb = consts.tile([128, 128], F32R)
        nc.vector.tensor_copy(wt_sb[:], wt_ps[:, 0:128])

        for c in range(N // 512):
            xin = ssb.tile([128, 512], F32, tag="xin", name=f"xin{c}")
            nc.sync.dma_start(
                xin[:].rearrange("p (t d) -> p t d", d=D),
                xk_r[:, 4 * c:4 * c + 4, :],
            )
            tp = sps.tile([128, 512], F32, tag="tp", bufs=2, name=f"tp{c}")
            for j in range(4):
                nc.tensor.transpose(tp[:, ts(j, 128)], xin[:, ts(j, 128)], ident[:])
            xt = ssb.tile([128, 512], F32R, tag="xt", name=f"xt{c}")
            nc.scalar.activation(xt[:], tp[:], AF.Identity)
            hps = sps.tile([128, 512], F32, tag="hps", bufs=2, name=f"hps{c}")
            nc.tensor.matmul(hps[:], wt_sb[:], xt[:], start=True, stop=True)
            nc.scalar.activation(hk[:, ts(c, 512)], hps[:], AF.Identity,
                                 bias=b_sb[:, 0:1])
            if c < NQ // 512:
                nc.scalar.activation(gq[:, ts(c, 512)], hps[:], AF.Identity,
                                     bias=bm2_sb[:, 0:1], scale=-2.0)
            s2f = ssb.tile([128, 512], F32R, tag="s2f", name=f"s2f{c}")
            nc.scalar.activation(s2f[:], hk[:, ts(c, 512)], AF.Square)
            bcps = sps.tile([128, 512], F32, tag="bcps", bufs=2, name=f"bcps{c}")
            nc.tensor.matmul(bcps[:], ones_full[:], s2f[:], start=True, stop=True)
            # |h_k|^2 broadcast; drains alternate DVE/Act to balance setup
            if c % 2 == 0:
                nc.vector.tensor_copy(sqk_bc[:, ts(c, 512)], bcps[:])
            else:
                nc.scalar.activation(sqk_bc[:, ts(c, 512)], bcps[:], AF.Identity)

        # per-partition query norms: transpose row 0 of the broadcast tile
        sqq_ps = sps.tile([128, 512], F32, tag="sqq", bufs=1, name="sqq_ps")
        for q in range(NQT):
            nc.tensor.transpose(sqq_ps[:, q:q + 1], sqk_bc[0:1, ts(q, 128)],
                                ident[0:1, 0:1])
        nc.vector.tensor_copy(sqq_cols[:], sqq_ps[:, 0:NQT])

    # ---------------- main loop ----------------
    stage = ctx.enter_context(tc.tile_pool(name="stage", bufs=3))
    obf_pool = ctx.enter_context(tc.tile_pool(name="obf", bufs=3))
    mm_ps = ctx.enter_context(tc.tile_pool(name="mm", bufs=4, space="PSUM"))

    for q in range(NQT):
        q0 = 128 * q
        lhs = gq[:, q0:q0 + 128]
        d2 = stage.tile([128, WQ], F32, tag="d2", name=f"d2_{q}")
        for off, w in ((0, 1024), (1024, 1024), (2048, 128)):
            pp = mm_ps.tile([128, 1024], F32, tag="pp", name=f"pp_{q}_{off}")
            for j in range(0, w, 512):
                wj = min(512, w - j)
                c0 = q0 + off + j
                nc.tensor.matmul(pp[:, j:j + wj], lhs, hk[:, c0:c0 + wj],
                                 start=True, stop=True)
            nc.vector.scalar_tensor_tensor(
                d2[:, off:off + w], pp[:, 0:w], sqq_cols[:, q:q + 1],
                sqk_bc[:, q0 + off:q0 + off + w], ALU.add, ALU.add,
            )
        nc.gpsimd.tensor_tensor(d2[:], d2[:], half_tile[:], ALU.pow)
        obf = obf_pool.tile([128, WQ], BF16, tag="obf", name=f"obf_{q}")
        nc.scalar.activation(obf[:], d2[:], AF.Exp, scale=-TEMPERATURE)
        nc.vector.scalar_tensor_tensor(
            obf[:, 0:128], obf[:, 0:128], 1.0, ident_bf[:], ALU.min, ALU.max,
        )
        nc.sync.dma_start(out[q0:q0 + 128, q0:q0 + WQ], obf[:])


def build_nc():
    nc = bacc.Bacc("TRN2", target_bir_lowering=False, debug=False)
    xk = nc.dram_tensor("xk", [N, D], F32, kind="ExternalInput").ap()
    W = nc.dram_tensor("W", [D, D], F32, kind="ExternalInput").ap()
    b = nc.dram_tensor("b", [D, 1], F32, kind="ExternalInput").ap()
    out = nc.dram_tensor("out", [NQ, N], BF16, kind="ExternalOutput").ap()
    with tile.TileContext(nc) as tc:
        with ExitStack() as ctx:
            kernel_body(ctx, tc, out, xk, W, b)
    nc.compile()
    return nc


_NC_CACHE = None


def _get_nc():
    global _NC_CACHE
    if _NC_CACHE is None:
        _NC_CACHE = build_nc()
    return _NC_CACHE


def _run(x, W, b, trace=False, **spmd_kwargs):
    from concourse.bass_utils import run_bass_kernel_spmd

    x = np.asarray(x, dtype=np.float32)
    W = np.asarray(W, dtype=np.float32)
    b = np.asarray(b, dtype=np.float32).reshape(D, 1)
    nc = _get_nc()
    in_maps = []
    for c in range(N_CORES):
        bi, hf = c // 2, c % 2
        xl = np.roll(x[bi], -NQ * hf, axis=0) if hf else x[bi]
        in_maps.append({"xk": np.ascontiguousarray(xl), "W": W, "b": b})
    res = run_bass_kernel_spmd(
        nc, in_maps, core_ids=list(range(N_CORES)), trace=trace, **spmd_kwargs
    )
    out = np.zeros((B, N, N), dtype=np.float32)
    for c in range(N_CORES):
        bi, hf = c // 2, c % 2
        loc = np.asarray(res.results[c]["out"]).astype(np.float32)
        base = NQ * hf
        for q in range(NQT):
            r0 = base + 128 * q
            cs = base + 128 * q      # global col start == global row start
            rows = out[bi, r0:r0 + 128]
            blk = loc[128 * q:128 * q + 128, 128 * q:128 * q + WQ]
            ce = cs + WQ
            if ce <= N:
                rows[:, cs:ce] = blk
            else:
                w1 = N - cs
                rows[:, cs:] = blk[:, :w1]
                rows[:, :ce - N] = blk[:, w1:]
    # mirror the residues the device did not compute: (col-row) mod 32 in
    # [17,31], each the transpose of a computed block with residue 32-d
    i = np.arange(32)
    for d in range(17, 32):
        cidx = (i + d) % 32
        for bi in range(B):
            B4 = out[bi].reshape(32, 128, 32, 128)
            B4[i, :, cidx, :] = B4[cidx, :, i, :].transpose(0, 2, 1)
    return out, res


def kernel(x, W, b):
    out, _ = _run(x, W, b)
    return out


# revision 7
# speedup vs baseline: 1.0793x; 1.0793x over previous
"""Self-contained TRN2 Bass kernel for nn_EuclideanSimilarity.

Full-input contract: kernel(x, W, b) with
  x [4, 4096, 128] f32, W [128, 128] f32, b [128] f32
returns out [4, 4096, 4096] f32 = exp(-pairwise_euclidean_dist(x @ W.T + b)).

The output is symmetric per batch, so each core computes only the circulant
upper half: query row-block i (128 rows) against key blocks i..i+16 (mod 32),
2176 columns per row-block. Every unordered pair is covered exactly once
(d = |i-j| mod 32 in [1,15] once, d=16 from both sides, d=0 the diagonal);
the host mirrors the remaining residues with block transposes at unshard
time. 8 cores: core c -> (batch c//2, half c%2); the host rotates the key
axis by 2048*half so all cores run an IDENTICAL program on local qtiles
0..15 with key columns [128q, 128q+2176) - no wraparound on device.

Per-core pipeline (engines balanced, ~one pass each):
  PE    gram matmuls, single fp32r pass (lhsT = -2h slice, rhs = h)
  DVE   fused drain: d2 = (psum + |h_q|^2) + |h_k|^2-broadcast, one
        scalar_tensor_tensor per PSUM tile (PSUM readable only by DVE/Act)
  Pool  d = d2^0.5 via tensor_tensor pow (GPSIMD cannot touch PSUM, but
        pow is only ISA-valid on GPSIMD - so the drain and the sqrt trade
        engines), in-place on the staging tile
  Act   out_bf16 = Exp(-d); Identity+Exp live in one activation-table set,
        so there are no mid-kernel table swaps
  DVE   diagonal-block fix: (out min 1.0) max I. fp32r noise makes the
        diagonal d2 ~ +-0.02 (sqrt of a negative -> NaN); min/max are IEEE
        minNum/maxNum on-chip, so NaN and any >1 garbage both collapse to
        the exact 1.0 the diagonal wants.
  DMA   one [128, 2176] bf16 store per qtile (~9.5 MB/core instead of 32)

The bf16 store + half-sized compute put every engine near ~30-45us; fp32r
gram noise (~2^-13) only matters on the diagonal, which the fix rewrites.
"""

from contextlib import ExitStack

import numpy as np

import concourse.mybir as mybir
import concourse.tile as tile
from concourse import bacc
from concourse.bass import ts
from concourse.masks import make_identity

F32 = mybir.dt.float32
F32R = mybir.dt.float32r
BF16 = mybir.dt.bfloat16
AF = mybir.ActivationFunctionType
ALU = mybir.AluOpType

B = 4
N = 4096
D = 128
NQ = 2048   # local query rows per core
NQT = 16    # local query tiles (128 rows each)
WQ = 2176   # key columns per qtile: 17 x 128 blocks
N_CORES = 8
TEMPERATURE = 1.0


def kernel_body(ctx: ExitStack, tc: tile.TileContext, out, xk, W, b):
    nc = tc.nc

    consts = ctx.enter_context(tc.tile_pool(name="consts", bufs=1))
    ident = consts.tile([128, 128], F32)
    make_identity(nc, ident[:])
    ident_bf = consts.tile([128, 128], BF16)
    nc.vector.tensor_copy(ident_bf[:], ident[:])

    w_sb = consts.tile([128, 128], F32)
    nc.sync.dma_start(w_sb[:], W[:, :])
    b_sb = consts.tile([128, 1], F32)
    nc.sync.dma_start(b_sb[:], b[:, :])
    bm2_sb = consts.tile([128, 1], F32)
    nc.scalar.mul(bm2_sb[:], b_sb[:], -2.0)

    ones_f = consts.tile([128, 128], F32)
    nc.gpsimd.memset(ones_f[:], 1.0)
    ones_full = consts.tile([128, 128], F32R)
    nc.vector.tensor_copy(ones_full[:], ones_f[:])

    half_tile = consts.tile([128, WQ], F32)
    nc.vector.memset(half_tile[:], 0.5)

    # persistent operands
    hpool = ctx.enter_context(tc.tile_pool(name="h", bufs=1))
    hk = hpool.tile([128, N], F32R)        # h keys, d-major
    gq = hpool.tile([128, NQ], F32R)       # -2h for local queries
    sqk_bc = hpool.tile([128, N], F32)     # |h_k|^2 broadcast across partitions
    sqq_cols = hpool.tile([128, NQT], F32)  # |h_q|^2, one col per qtile

    xk_r = xk.rearrange("(t p) d -> p t d", p=128)

    # ---------------- setup: project keys, norms ----------------
    with tc.tile_pool(name="setup_sb", bufs=3) as ssb, \
         tc.tile_pool(name="setup_ps", bufs=2, space="PSUM") as sps:

        wt_ps = sps.tile([128, 512], F32, tag="sqq", bufs=2)
        nc.tensor.transpose(wt_ps[:, 0:128], w_sb[:], ident[:])
        wt_sb = consts.tile([128, 128], F32R)
        nc.vector.tensor_copy(wt_sb[:], wt_ps[:, 0:128])

        for c in range(N // 512):
            xin = ssb.tile([128, 512], F32, tag="xin", name=f"xin{c}")
            nc.sync.dma_start(
                xin[:].rearrange("p (t d) -> p t d", d=D),
                xk_r[:, 4 * c:4 * c + 4, :],
            )
            tp = sps.tile([128, 512], F32, tag="tp", bufs=2, name=f"tp{c}")
            for j in range(4):
                nc.tensor.transpose(tp[:, ts(j, 128)], xin[:, ts(j, 128)], ident[:])
            xt = ssb.tile([128, 512], F32R, tag="xt", name=f"xt{c}")
            nc.scalar.activation(xt[:], tp[:], AF.Identity)
            hps = sps.tile([128, 512], F32, tag="hps", bufs=2, name=f"hps{c}")
            nc.tensor.matmul(hps[:], wt_sb[:], xt[:], start=True, stop=True)
            nc.scalar.activation(hk[:, ts(c, 512)], hps[:], AF.Identity,
                                 bias=b_sb[:, 0:1])
            if c < NQ // 512:
                nc.scalar.activation(gq[:, ts(c, 512)], hps[:], AF.Identity,
                                     bias=bm2_sb[:, 0:1], scale=-2.0)
            s2f = ssb.tile([128, 512], F32R, tag="s2f", name=f"s2f{c}")
            nc.vector.tensor_tensor(s2f[:], hk[:, ts(c, 512)], hk[:, ts(c, 512)],
                                    ALU.mult)
            bcps = sps.tile([128, 512], F32, tag="bcps", bufs=2, name=f"bcps{c}")
            nc.tensor.matmul(bcps[:], ones_full[:], s2f[:], start=True, stop=True)
            nc.vector.tensor_copy(sqk_bc[:, ts(c, 512)], bcps[:])
            if c < 4:
                # per-partition query norms: transpose row 0 of the broadcast
                # tile chunk-by-chunk so drains can start at half-setup
                sqq_ps = sps.tile([128, 512], F32, tag="sqq", name=f"sqqps{c}")
                for j in range(4):
                    nc.tensor.transpose(sqq_ps[:, j:j + 1],
                                        sqk_bc[0:1, ts(4 * c + j, 128)],
                                        ident[0:1, 0:1])
                nc.vector.tensor_copy(sqq_cols[:, 4 * c:4 * c + 4],
                                      sqq_ps[:, 0:4])

    # ---------------- main loop ----------------
    stage = ctx.enter_context(tc.tile_pool(name="stage", bufs=3))
    obf_pool = ctx.enter_context(tc.tile_pool(name="obf", bufs=3))
    mm_ps = ctx.enter_context(tc.tile_pool(name="mm", bufs=4, space="PSUM"))

    for q in range(NQT):
        q0 = 128 * q
        lhs = gq[:, q0:q0 + 128]
        d2 = stage.tile([128, WQ], F32, tag="d2", name=f"d2_{q}")
        for off, w in ((0, 1024), (1024, 1024), (2048, 128)):
            pp = mm_ps.tile([128, 1024], F32, tag="pp", name=f"pp_{q}_{off}")
            for j in range(0, w, 512):
                wj = min(512, w - j)
                c0 = q0 + off + j
                nc.tensor.matmul(pp[:, j:j + wj], lhs, hk[:, c0:c0 + wj],
                                 start=True, stop=True)
            nc.vector.scalar_tensor_tensor(
                d2[:, off:off + w], pp[:, 0:w], sqq_cols[:, q:q + 1],
                sqk_bc[:, q0 + off:q0 + off + w], ALU.add, ALU.add,
            )
        nc.gpsimd.tensor_tensor(d2[:], d2[:], half_tile[:], ALU.pow)
        obf = obf_pool.tile([128, WQ], BF16, tag="obf", name=f"obf_{q}")
        nc.scalar.activation(obf[:], d2[:], AF.Exp, scale=-TEMPERATURE)
        nc.vector.scalar_tensor_tensor(
            obf[:, 0:128], obf[:, 0:128], 1.0, ident_bf[:], ALU.min, ALU.max,
        )
        nc.sync.dma_start(out[q0:q0 + 128, q0:q0 + WQ], obf[:])


def build_nc():
    nc = bacc.Bacc("TRN2", target_bir_lowering=False, debug=False)
    xk = nc.dram_tensor("xk", [N, D], F32, kind="ExternalInput").ap()
    W = nc.dram_tensor("W", [D, D], F32, kind="ExternalInput").ap()
    b = nc.dram_tensor("b", [D, 1], F32, kind="ExternalInput").ap()
    out = nc.dram_tensor("out", [NQ, N], BF16, kind="ExternalOutput").ap()
    with tile.TileContext(nc) as tc:
        with ExitStack() as ctx:
            kernel_body(ctx, tc, out, xk, W, b)
    nc.compile()
    return nc


_NC_CACHE = None


def _get_nc():
    global _NC_CACHE
    if _NC_CACHE is None:
        _NC_CACHE = build_nc()
    return _NC_CACHE


def _run(x, W, b, trace=False, **spmd_kwargs):
    from concourse.bass_utils import run_bass_kernel_spmd

    x = np.asarray(x, dtype=np.float32)
    W = np.asarray(W, dtype=np.float32)
    b = np.asarray(b, dtype=np.float32).reshape(D, 1)
    nc = _get_nc()
    in_maps = []
    for c in range(N_CORES):
        bi, hf = c // 2, c % 2
        xl = np.roll(x[bi], -NQ * hf, axis=0) if hf else x[bi]
        in_maps.append({"xk": np.ascontiguousarray(xl), "W": W, "b": b})
    res = run_bass_kernel_spmd(
        nc, in_maps, core_ids=list(range(N_CORES)), trace=trace, **spmd_kwargs
    )
    out = np.zeros((B, N, N), dtype=np.float32)
    for c in range(N_CORES):
        bi, hf = c // 2, c % 2
        loc = np.asarray(res.results[c]["out"]).astype(np.float32)
        base = NQ * hf
        for q in range(NQT):
            r0 = base + 128 * q
            cs = base + 128 * q      # global col start == global row start
            rows = out[bi, r0:r0 + 128]
            blk = loc[128 * q:128 * q + 128, 128 * q:128 * q + WQ]
            ce = cs + WQ
            if ce <= N:
                rows[:, cs:ce] = blk
            else:
                w1 = N - cs
                rows[:, cs:] = blk[:, :w1]
                rows[:, :ce - N] = blk[:, w1:]
    # mirror the residues the device did not compute: (col-row) mod 32 in
    # [17,31], each the transpose of a computed block with residue 32-d
    i = np.arange(32)
    for d in range(17, 32):
        cidx = (i + d) % 32
        for bi in range(B):
            B4 = out[bi].reshape(32, 128, 32, 128)
            B4[i, :, cidx, :] = B4[cidx, :, i, :].transpose(0, 2, 1)
    return out, res


def kernel(x, W, b):
    out, _ = _run(x, W, b)
    return out
